# revision 54
# baseline (speedup 1.0000x reference)
"""Deep-MMD loss kernel for Trainium2, sharded across 8 NeuronCores.

Strategy (data-parallel row sharding, per the sharding hint):
  - Each core owns a 512-row block of X (and the same-index block of Y) and
    computes its row-blocks of the three 4096x4096 gram matrices
    k_x, k_y, k_xy fully fused on-chip (never materialized to HBM):
        k = exp(-(d_feat/sigma_phi + d_org/sigma_q))
  - Feature distances use the factorization  F = h3 @ W4 (+b4), so
        d_feat = (h3_i - h3_j)^T G (h3_i - h3_j),  G = W4 W4^T  (b4 cancels).
    With L = chol(G), w = 32*sqrt(2/sph)*L^T h3 (10 rows) centered by a
    per-component mean (distances are shift-invariant, so each core may use
    its own mean), the Exp exponent is assembled as
        E_ij = -2/sq * pk_ij - u_i,   pk = -w_i.w_j - x_i.x_j + 1024*u_j
    where u = |w|^2/2048 + xon/sq (= vn + xon/sq, the combined row norm).
    The w cross products run as ONE bf16 K=42 matmul per 512-col strip via
    an error-compensated hi/lo split
        w_i.w_j = hi_i.hi_j + hi_i.lo_j + lo_i.hi_j + lo_i.lo_j
    (all four terms carried -- K is free since matmul cost only depends on
    the streamed columns).  The 32x / 1024x scalings are exact powers of
    two (sq = 2048), undone by the Exp activation's scale=-2/sq; u's 1024x
    enters as an exact bf16 exponent shift on its hi/lo rows.
  - k_x and k_y are symmetric: with a per-core CYCLIC block permutation of
    the columns, each core computes only the 5 blocks at cyclic distance
    d=0..4 (2560 of 4096 cols).  The full sum is s0 + 2*s1(d1..3) + s2(d4)
    and the missing row-sum parts are other cores' transposed column sums
    (csx/csy), assembled on host.  k_xy is not symmetric and runs full.
  - Precision placement (f32r matmul noise measured at ~1.5e-4 relative):
    the MLP, the w-producing matmul, and the |w|^2 row-sum run in exact
    fp32 (2-pass); only the org cross products (O(1) values, tolerance
    ~1e-2 absolute in do) and the xon sums use 1-pass f32r.
  - Row sums fall out of the Exp activation's accum_out for free; the Exp
    writes bf16 so k_xy column sums are a ones-vector matmul with no cast;
    the diagonal (trace) is extracted from the un-rounded PSUM exponent.
  - The eps = sigmoid(epsilon_opt) ~ 5e-11 mixture term contributes
    ~3e-16 to mmd2 (measured in f64) and is dropped.
  - Host (float64) assembles the final [mmd2, var] from per-core partial
    sums ("all-reduce the scalar sums" per the hint).

SPMD trick: every core's column order is cyclically permuted "own block
first" (host-side input prep), so its diagonal always lives in columns
[c*128,(c+1)*128) of the first 512-col block -- the compiled program is
identical on all 8 cores; only input data differs.

Measured: 254906 ns on TRN2 (baseline 674563 ns), rel err 6.5e-4 vs the
f64 oracle (gate 2e-2).
"""

import numpy as np

N = 4096          # samples per side
IN_DIM = 256
HID = 10
NCORES = 8
BLK = N // NCORES           # 512 rows per core
NCH = BLK // 128            # 4 row-chunks of 128 per core
NST = N // 1024             # 4 column supertiles of 1024
SW = 64 + HID               # 74: stacked block 0 at partitions 0:10, block 1 at 64:74
KU = 42                     # U rows: hi lo hi lo (10 each), t_hi, t_lo
NEG2SQ = -1.0 / 1024.0      # -2/sq with sq = 2048 (exact binary)
O2048 = 1.0 / 2048.0        # norm-sum lhs constant (exact binary)


def _build_bass():
    import concourse.bass as bass  # noqa: F401
    import concourse.mybir as mybir
    import concourse.tile as tile
    from concourse import bacc

    f32 = mybir.dt.float32
    f32r = mybir.dt.float32r
    bf16 = mybir.dt.bfloat16
    AFT = mybir.ActivationFunctionType

    nc = bacc.Bacc("TRN2")

    # ---------------- DRAM I/O ----------------
    # One copy of the inputs, declared f32r so the BIR verifier accepts the
    # gram-phase f32r matmul consumers; the MLP reads the same SBUF tiles
    # through a f32 bitcast (the DMA write is a byte copy, so full-precision
    # f32 bits flow to the fp32 matmuls either way).
    xt = nc.dram_tensor("xt", [IN_DIM, N], f32r, kind="ExternalInput")
    yt = nc.dram_tensor("yt", [IN_DIM, N], f32r, kind="ExternalInput")
    xbts = nc.dram_tensor("xbts", [IN_DIM, BLK], f32r, kind="ExternalInput")
    ybts = nc.dram_tensor("ybts", [IN_DIM, BLK], f32r, kind="ExternalInput")
    w1 = nc.dram_tensor("w1", [IN_DIM, HID], f32, kind="ExternalInput")
    w2b = nc.dram_tensor("w2b", [SW, SW], f32, kind="ExternalInput")
    w3b = nc.dram_tensor("w3b", [SW, SW], f32, kind="ExternalInput")
    lvs2 = nc.dram_tensor("lvs2", [SW, 42], f32, kind="ExternalInput")
    b1s2 = nc.dram_tensor("b1s2", [SW, 1], f32, kind="ExternalInput")
    b2s = nc.dram_tensor("b2s", [SW, 1], f32, kind="ExternalInput")
    b3s = nc.dram_tensor("b3s", [SW, 1], f32, kind="ExternalInput")
    wsum = nc.dram_tensor("wsum", [42, 2], f32, kind="ExternalInput")
    # xon/sq per permuted column, host-computed, already in the t128
    # layout (t128[p, c] = val[32p + c])
    xont = nc.dram_tensor("xont", [128, 32], f32, kind="ExternalInput")
    yont = nc.dram_tensor("yont", [128, 32], f32, kind="ExternalInput")
    onesc = nc.dram_tensor("onesc", [128, 1], bf16, kind="ExternalInput")
    ones2 = nc.dram_tensor("ones2", [2, BLK], bf16, kind="ExternalInput")
    eye = nc.dram_tensor("eye", [128, 128], f32, kind="ExternalInput")

    # Triangle outputs for the symmetric k_x/k_y: per row-chunk 3 accum
    # slots (s0 = cyclic-distance-0 block, s1 = d1..d3, s2 = d4), plus the
    # column sums of the d1..d3 blocks for the host's transposed row sums.
    rsx = nc.dram_tensor("rsx", [128, NCH * 3], f32, kind="ExternalOutput")
    rsy = nc.dram_tensor("rsy", [128, NCH * 3], f32, kind="ExternalOutput")
    csx = nc.dram_tensor("csx", [1, 1536], f32, kind="ExternalOutput")
    csy = nc.dram_tensor("csy", [1, 1536], f32, kind="ExternalOutput")
    rsxy = nc.dram_tensor("rsxy", [128, NCH * NST], f32, kind="ExternalOutput")
    csxy = nc.dram_tensor("csxy", [1, N], f32, kind="ExternalOutput")
    dgx = nc.dram_tensor("dgx", [128, NCH], f32, kind="ExternalOutput")
    dgy = nc.dram_tensor("dgy", [128, NCH], f32, kind="ExternalOutput")
    dgxy = nc.dram_tensor("dgxy", [128, NCH], f32, kind="ExternalOutput")

    with tile.TileContext(nc) as tc:
        with tc.tile_pool(name="persist", bufs=1) as pp:
            # ---------- SBUF (persistent) ----------
            t_x = [pp.tile([128, N], f32r, name=f"x{i}", tag=f"x{i}")
                   for i in range(2)]
            t_y = [pp.tile([128, N], f32r, name=f"y{i}", tag=f"y{i}")
                   for i in range(2)]
            t_xbts = [pp.tile([128, BLK], f32r, name=f"xbts{i}", tag=f"xbts{i}")
                      for i in range(2)]
            t_ybts = [pp.tile([128, BLK], f32r, name=f"ybts{i}", tag=f"ybts{i}")
                      for i in range(2)]
            urx = pp.tile([KU, N], bf16, name="urx", tag="urx")
            ury = pp.tile([KU, N], bf16, name="ury", tag="ury")
            ulx = pp.tile([KU, BLK], bf16, name="ulx", tag="ulx")
            uly = pp.tile([KU, BLK], bf16, name="uly", tag="uly")
            t_w1 = [pp.tile([128, HID], f32, name=f"w1{i}", tag=f"w1{i}")
                    for i in range(2)]
            t_w2b = pp.tile([SW, SW], f32, name="w2b", tag="w2b")
            t_w3b = pp.tile([SW, SW], f32, name="w3b", tag="w3b")
            t_lvs2 = pp.tile([SW, 42], f32, name="lvs2", tag="lvs2")
            t_b1s2 = pp.tile([SW, 1], f32, name="b1s2", tag="b1s2")
            t_b2s = pp.tile([SW, 1], f32, name="b2s", tag="b2s")
            t_b3s = pp.tile([SW, 1], f32, name="b3s", tag="b3s")
            t_wsum = pp.tile([42, 2], f32, name="wsum", tag="wsum")
            t_xont = pp.tile([128, 32], f32, name="xont", tag="xont")
            t_yont = pp.tile([128, 32], f32, name="yont", tag="yont")
            t_ones = pp.tile([128, 1], bf16, name="ones", tag="ones")
            t_eye = pp.tile([128, 128], f32, name="eye", tag="eye")
            cnbx = pp.tile([128, NCH], f32, name="cnbx", tag="cnbx")
            cnby = pp.tile([128, NCH], f32, name="cnby", tag="cnby")
            t128x = pp.tile([128, 32], f32, name="t128x", tag="t128x")
            t128y = pp.tile([128, 32], f32, name="t128y", tag="t128y")
            t_rs = {m: pp.tile([128, NCH * 3], f32, name=f"rs{m}", tag=f"rs{m}")
                    for m in "xy"}
            t_rs["z"] = pp.tile([128, NCH * NST], f32, name="rsz", tag="rsz")
            t_dg = {m: pp.tile([128, NCH], f32, name=f"dg{m}", tag=f"dg{m}")
                    for m in "xyz"}

            # ---------- input DMAs ----------
            # Priority order: MLP weights, then x then y chunks (phase B
            # consumes x first); gram-only tensors (xbts, eye, ones) last.
            # The sync queue runs transfers serially, so order = latency.
            for half in range(2):
                nc.sync.dma_start(t_w1[half][:],
                                  w1[half * 128:(half + 1) * 128, :])
            nc.sync.dma_start(t_w2b[:], w2b[:])
            nc.sync.dma_start(t_w3b[:], w3b[:])
            nc.sync.dma_start(t_lvs2[:], lvs2[:])
            nc.sync.dma_start(t_b1s2[:], b1s2[:])
            nc.sync.dma_start(t_b2s[:], b2s[:])
            nc.sync.dma_start(t_b3s[:], b3s[:])
            nc.sync.dma_start(t_wsum[:], wsum[:])
            nc.sync.dma_start(t_xont[:], xont[:])
            nc.sync.dma_start(t_yont[:], yont[:])
            for tt, src in ((t_x, xt), (t_y, yt)):
                for j in range(8):
                    s = slice(j * 512, (j + 1) * 512)
                    for half in range(2):
                        hs_ = slice(half * 128, (half + 1) * 128)
                        nc.sync.dma_start(tt[half][:, s], src[hs_, s])
            for half in range(2):
                hs_ = slice(half * 128, (half + 1) * 128)
                nc.sync.dma_start(t_xbts[half][:], xbts[hs_, :])
                nc.sync.dma_start(t_ybts[half][:], ybts[hs_, :])
            nc.sync.dma_start(t_ones[:], onesc[:])
            nc.sync.dma_start(t_eye[:], eye[:])
            nc.sync.dma_start(ulx[40:42, :], ones2[:])
            nc.sync.dma_start(uly[40:42, :], ones2[:])

            # ---------- Phase B: MLP + w + U assembly + norms ----------
            # softplus(z) = Ln(Exp(z) + 1): no HW softplus table,
            # but ln+exp share one table set.
            if True:
                with tc.tile_pool(name="mlp_ps", bufs=2, space="PSUM") as mps, \
                     tc.tile_pool(name="cn_ps", bufs=2, space="PSUM") as cnps, \
                     tc.tile_pool(name="hp", bufs=1) as hp, \
                     tc.tile_pool(name="ep", bufs=1) as ep, \
                     tc.tile_pool(name="sp", bufs=1) as sp:
                    hh01 = [hp.tile([SW, 2048], f32, name=f"h{l}", tag=f"h{l}")
                            for l in range(2)]
                    t_mneg = sp.tile([42, 1], f32, name="mneg", tag="mneg")
                    for t_in, ur, ul, t128, sname in (
                            (t_x, urx, ulx, t128x, "x"),
                            (t_y, ury, uly, t128y, "y")):
                        # h1 hole rows stay 0 (L2 contracts them against
                        # zero weights; garbage could be NaN)
                        hh = [hh01[0], hh01[1], hh01[0]]  # h3 reuses h1's buf
                        nc.vector.memset(hh[0][:], 0.0)
                        # L1: even blocks -> psum [10,2048] -> h1[0:10,:],
                        #     odd blocks  -> psum [10,2048] -> h1[64:74,:]
                        for par in range(2):
                            for g in range(2):
                                p1 = mps.tile([HID, 1024], f32, name="p1",
                                              tag="mp")
                                for qq in range(2):
                                    b = 2 * (2 * g + qq) + par
                                    s = slice(b * 512, (b + 1) * 512)
                                    po = p1[:, qq * 512:(qq + 1) * 512]
                                    nc.tensor.matmul(po, t_w1[0][:],
                                                     t_in[0][:, s].bitcast(f32),
                                                     start=True, stop=False)
                                    nc.tensor.matmul(po, t_w1[1][:],
                                                     t_in[1][:, s].bitcast(f32),
                                                     start=False, stop=True)
                                dst = hh[0][64 * par:64 * par + HID,
                                            g * 1024:(g + 1) * 1024]
                                e1 = ep.tile([HID, 1024], f32, name="e1",
                                             tag="e1")
                                nc.scalar.activation(e1[:], p1[:], AFT.Exp,
                                                     bias=t_b1s2[0:HID, :])
                                nc.scalar.activation(dst, e1[:], AFT.Ln,
                                                     bias=1.0)
                        # L2, L3: block-diagonal stacked
                        for wt, bt, hsrc, hdst in ((t_w2b, t_b2s, hh[0], hh[1]),
                                                   (t_w3b, t_b3s, hh[1], hh[2])):
                            for g in range(2):
                                pL = mps.tile([SW, 1024], f32, name="pL",
                                              tag="mp")
                                for qq in range(2):
                                    sq_ = slice(g * 1024 + qq * 512,
                                                g * 1024 + qq * 512 + 512)
                                    nc.tensor.matmul(
                                        pL[:, qq * 512:(qq + 1) * 512],
                                        wt[:], hsrc[:, sq_],
                                        start=True, stop=True)
                                eL = ep.tile([SW, 1024], f32, name="ea",
                                             tag="ea")
                                nc.scalar.activation(eL[:], pL[:], AFT.Exp,
                                                     bias=bt[:])
                                nc.scalar.activation(
                                    hdst[:, g * 1024:(g + 1) * 1024], eL[:],
                                    AFT.Ln, bias=1.0)
                        # w = lvs^T @ h3 per q (cols 2q*512 even / odd
                        # stacked as rows 0:10 / 32:42), centered, split
                        # hi/lo bf16, scattered into UR/UL; |w|^2 and xon
                        # sums accumulate u = vn + xon/sq per 512-block.
                        for q in range(4):
                            pv = mps.tile([42, 512], f32, name="pv", tag="mp")
                            nc.tensor.matmul(pv[:], t_lvs2[:],
                                             hh[2][:, q * 512:(q + 1) * 512],
                                             start=True, stop=True)
                            if sname == "x" and q == 0:
                                nc.vector.reduce_sum(
                                    t_mneg[:], pv[:], axis=mybir.AxisListType.X)
                                nc.vector.tensor_scalar_mul(
                                    t_mneg[:], t_mneg[:], 1.0 / 512.0)
                                # both stacked blocks share ONE mean
                                nc.gpsimd.dma_start(t_mneg[32:42, :],
                                                    t_mneg[0:10, :])
                            wq = sp.tile([42, 512], f32, name="wq", tag="wq",
                                         bufs=2)
                            nc.vector.tensor_scalar_sub(wq[:], pv[:], t_mneg[:])
                            hi16 = sp.tile([42, 512], bf16, name="hi16",
                                           tag="hi16", bufs=2)
                            lo16 = sp.tile([42, 512], bf16, name="lo16",
                                           tag="lo16", bufs=2)
                            nc.vector.tensor_copy(hi16[:], wq[:])
                            nc.vector.tensor_sub(lo16[:], wq[:], hi16[:])
                            w2q = sp.tile([42, 512], f32, name="w2q",
                                          tag="w2q", bufs=2)
                            nc.scalar.activation(w2q[:], wq[:], AFT.Square)
                            for par in range(2):
                                b = 2 * q + par
                                s = slice(b * 512, (b + 1) * 512)
                                r0 = 32 * par
                                nc.gpsimd.dma_start(ur[0:10, s],
                                                    hi16[r0:r0 + 10, :])
                                nc.gpsimd.dma_start(ur[10:20, s],
                                                    lo16[r0:r0 + 10, :])
                                nc.gpsimd.dma_start(ur[20:30, s],
                                                    hi16[r0:r0 + 10, :])
                                nc.gpsimd.dma_start(ur[30:40, s],
                                                    lo16[r0:r0 + 10, :])
                                if b == 0:
                                    nhi = sp.tile([HID, 512], bf16, name="nhi",
                                                  tag="nhi")
                                    nlo = sp.tile([HID, 512], bf16, name="nlo",
                                                  tag="nlo")
                                    nc.vector.tensor_scalar_mul(
                                        nhi[:], hi16[0:10, :], -1.0)
                                    nc.vector.tensor_scalar_mul(
                                        nlo[:], lo16[0:10, :], -1.0)
                                    nc.gpsimd.dma_start(ul[0:10, :], nhi[:])
                                    nc.gpsimd.dma_start(ul[10:20, :], nhi[:])
                                    nc.gpsimd.dma_start(ul[20:30, :], nlo[:])
                                    nc.gpsimd.dma_start(ul[30:40, :], nlo[:])
                                # vn = |w|^2/2048 per column; xon/sq is
                                # host-provided and added in the tail
                                cnp = cnps.tile([1, 512], f32, name="cnp",
                                                tag="cnp")
                                nc.tensor.matmul(cnp[:],
                                                 t_wsum[:, par:par + 1],
                                                 w2q[:],
                                                 start=True, stop=True)
                                cnrow = sp.tile([1, 512], f32, name="cnrow",
                                                tag="cnrow", bufs=2)
                                nc.scalar.copy(cnrow[:], cnp[:])
                                nc.gpsimd.dma_start(
                                    t128[16 * b:16 * b + 16, :], cnrow[:])

            # ---------- Phase C tail: u hi/lo rows + Exp bias ----------
            # t128[p, c] = u[32*p + c]; every row <-> t128 transfer uses the
            # same DMA linearization, so elementwise ops and the chunk
            # extraction (chunk c = t128[4c:4c+4, :]) stay consistent.
            # UR rows carry 1024*u as exact bf16 exponent shifts.
            with tc.tile_pool(name="tp", bufs=1) as tp:
                for sname, ur, t128, t_on, cnb in (
                        ("x", urx, t128x, t_xont, cnbx),
                        ("y", ury, t128y, t_yont, cnby)):
                    usum = tp.tile([128, 32], f32, name="usum", tag="usum")
                    uhi = tp.tile([128, 32], bf16, name="uhi", tag="uhi")
                    uhi32 = tp.tile([128, 32], f32, name="uhi32", tag="uhi32")
                    ulo32 = tp.tile([128, 32], f32, name="ulo32", tag="ulo32")
                    thi = tp.tile([128, 32], bf16, name="thi", tag="thi")
                    tlo = tp.tile([128, 32], bf16, name="tlo", tag="tlo")
                    nc.vector.tensor_add(usum[:], t128[:], t_on[:])
                    nc.vector.tensor_copy(uhi[:], usum[:])
                    nc.vector.tensor_copy(uhi32[:], uhi[:])
                    nc.vector.tensor_sub(ulo32[:], usum[:], uhi32[:])
                    nc.vector.tensor_scalar_mul(thi[:], uhi[:], 1024.0)
                    nc.vector.tensor_scalar_mul(tlo[:], ulo32[:], 1024.0)
                    nc.gpsimd.dma_start(ur[40:41, :], thi[:])
                    nc.gpsimd.dma_start(ur[41:42, :], tlo[:])
                    for c in range(NCH):
                        nc.gpsimd.dma_start(cnb[:, c:c + 1],
                                            usum[4 * c:4 * c + 4, :])
                    nc.vector.tensor_scalar_mul(cnb[:], cnb[:], -1.0)

            # ---------- Phase D1: k_x / k_y triangle (cols 0:2560) ----------
            # Cyclic column permutation means the 5 leading 512-col blocks
            # are cyclic distances d=0..4 from the own row block; d=5..7 are
            # recovered on host from other cores' transposes (k symmetric).
            with tc.tile_pool(name="kp", bufs=2) as kp, \
                 tc.tile_pool(name="gps", bufs=1, space="PSUM") as gps, \
                 tc.tile_pool(name="csbp", bufs=2) as csbp, \
                 tc.tile_pool(name="dp", bufs=3) as dp:
                mats = [
                    ("x", t_xbts, t_x, ulx, urx, cnbx, rsx, dgx, csx),
                    ("y", t_ybts, t_y, uly, ury, cnby, rsy, dgy, csy),
                ]
                for (mkey, bts, ct, ul, ur, cnb, rs_d, dg_d, cs_d) in mats:
                    rs_t, dg_t = t_rs[mkey], t_dg[mkey]
                    csp = gps.tile([1, 1536], f32, name="csp", tag="csp")
                    for c in range(NCH):
                        cs_ = slice(c * 128, (c + 1) * 128)
                        pks = [gps.tile([128, 512], f32, name="pk0", tag="pk0"),
                               gps.tile([128, 1536], f32, name="pk1", tag="pk1"),
                               gps.tile([128, 512], f32, name="pk2", tag="pk2")]
                        for s5 in range(5):
                            ns = slice(s5 * 512, (s5 + 1) * 512)
                            if s5 == 0:
                                po = pks[0][:]
                            elif s5 < 4:
                                po = pks[1][:, (s5 - 1) * 512:s5 * 512]
                            else:
                                po = pks[2][:]
                            nc.tensor.matmul(
                                po, bts[0][:, cs_], ct[0][:, ns],
                                start=True, stop=False)
                            nc.tensor.matmul(
                                po, bts[1][:, cs_], ct[1][:, ns],
                                start=False, stop=False)
                            nc.tensor.matmul(
                                po, ul[:, cs_], ur[:, ns],
                                start=False, stop=True)
                        kts = []
                        for si, pk in enumerate(pks):
                            kt = kp.tile([128, pk.shape[1]], bf16,
                                         name=f"kt{si}", tag=f"kt{si}")
                            nc.scalar.activation(
                                kt[:], pk[:], AFT.Exp, scale=NEG2SQ,
                                bias=cnb[:, c:c + 1],
                                accum_out=rs_t[:, c * 3 + si:c * 3 + si + 1])
                            kts.append(kt)
                        for k3 in range(3):
                            nc.tensor.matmul(
                                csp[:, k3 * 512:(k3 + 1) * 512],
                                t_ones[:],
                                kts[1][:, k3 * 512:(k3 + 1) * 512],
                                start=(c == 0), stop=(c == NCH - 1))
                        # diag from the un-rounded PSUM exponent (own block)
                        dtmp = dp.tile([128, 128], f32, name="dtmp", tag="dtmp")
                        ez = dp.tile([128, 1], f32, name="ez", tag="ez")
                        nc.vector.tensor_mul(dtmp[:], pks[0][:, cs_], t_eye[:])
                        nc.vector.reduce_sum(
                            ez[:], dtmp[:], axis=mybir.AxisListType.X)
                        nc.scalar.activation(
                            dg_t[:, c:c + 1], ez[:], AFT.Exp,
                            scale=NEG2SQ, bias=cnb[:, c:c + 1])
                    csrow = csbp.tile([1, 1536], f32, name="csrow", tag="csrow")
                    nc.scalar.copy(csrow[:], csp[:])
                    nc.sync.dma_start(cs_d[:], csrow[:])
                    nc.sync.dma_start(rs_d[:], rs_t[:])
                    nc.sync.dma_start(dg_d[:], dg_t[:])

            # ---------- Phase D2: k_xy (full 4096 cols) ----------
            with tc.tile_pool(name="kpz", bufs=4) as kpz, \
                 tc.tile_pool(name="gpsz", bufs=3, space="PSUM") as gpsz, \
                 tc.tile_pool(name="csps", bufs=1, space="PSUM") as csps, \
                 tc.tile_pool(name="csbpz", bufs=2) as csbpz, \
                 tc.tile_pool(name="dpz", bufs=3) as dpz:
                rs_t, dg_t = t_rs["z"], t_dg["z"]
                for j in range(NST):
                    csp = csps.tile([1, 1024], f32, name="cspz", tag="cspz")
                    for c in range(NCH):
                        cs_ = slice(c * 128, (c + 1) * 128)
                        pk = gpsz.tile([128, 1024], f32, name="pk", tag="pk")
                        for nh in range(2):
                            ns = slice(j * 1024 + nh * 512,
                                       j * 1024 + nh * 512 + 512)
                            po = pk[:, nh * 512:(nh + 1) * 512]
                            nc.tensor.matmul(
                                po, t_xbts[0][:, cs_], t_y[0][:, ns],
                                start=True, stop=False)
                            nc.tensor.matmul(
                                po, t_xbts[1][:, cs_], t_y[1][:, ns],
                                start=False, stop=False)
                            nc.tensor.matmul(
                                po, ulx[:, cs_], ury[:, ns],
                                start=False, stop=True)
                        kt = kpz.tile([128, 1024], bf16, name="kt", tag="kt")
                        nc.scalar.activation(
                            kt[:], pk[:], AFT.Exp, scale=NEG2SQ,
                            bias=cnbx[:, c:c + 1],
                            accum_out=rs_t[:, c * NST + j:c * NST + j + 1])
                        for nh in range(2):
                            nc.tensor.matmul(
                                csp[:, nh * 512:(nh + 1) * 512],
                                t_ones[:],
                                kt[:, nh * 512:(nh + 1) * 512],
                                start=(c == 0), stop=(c == NCH - 1))
                        if j == 0:
                            dtmp = dpz.tile([128, 128], f32, name="dtmp",
                                            tag="dtmp")
                            ez = dpz.tile([128, 1], f32, name="ez", tag="ez")
                            nc.vector.tensor_mul(dtmp[:], pk[:, cs_], t_eye[:])
                            nc.vector.reduce_sum(
                                ez[:], dtmp[:], axis=mybir.AxisListType.X)
                            nc.scalar.activation(
                                dg_t[:, c:c + 1], ez[:], AFT.Exp,
                                scale=NEG2SQ, bias=cnbx[:, c:c + 1])
                    csrow = csbpz.tile([1, 1024], f32, name="csrowz",
                                       tag="csrowz")
                    nc.scalar.copy(csrow[:], csp[:])
                    nc.sync.dma_start(
                        csxy[:, j * 1024:(j + 1) * 1024], csrow[:])
                nc.sync.dma_start(rsxy[:], rs_t[:])
                nc.sync.dma_start(dgxy[:], dg_t[:])

    # Force a single activation table set (ln+exp+square+copy all live in
    # natural_log_exp_and_others); the default greedy choice ping-pongs
    # between exp_and_others and natural_log, costing a ~1.3us table load
    # per switch.
    tabs = bacc.get_activation_tables(nc.m.arch)
    only = {name: (funcs if name == "natural_log_exp_and_others" else set())
            for name, funcs in tabs.items()}
    orig_fn = bacc.get_activation_tables
    bacc.get_activation_tables = lambda arch: only
    try:
        nc.compile()
    finally:
        bacc.get_activation_tables = orig_fn
    return nc


_NC_CACHE = None
_LAST_RESULT = None


def _harden_tracing():
    """Make run_bass_kernel_spmd(trace=True / BASS_TRACE=1) survive in
    containers whose antenv package lacks axon_hooks, and whose bucket
    upload is unavailable. No-ops when everything is present."""
    import sys
    import types
    try:
        import antenv.axon_hooks  # noqa: F401
    except ImportError:
        mod = types.ModuleType("antenv.axon_hooks")
        mod._hook = None
        mod.set_axon_ntff_profile_hook = lambda h: setattr(mod, "_hook", h)
        mod.get_axon_ntff_profile_hook = lambda: mod._hook
        sys.modules["antenv.axon_hooks"] = mod
        try:
            import antenv
            antenv.axon_hooks = mod
        except ImportError:
            pass
        try:
            from trn_agent_boot.trn_boot import _ntff_profile_via_ctypes
            hook = _ntff_profile_via_ctypes("/opt/axon/libaxon_pjrt.so")
            if hook is not None:
                mod.set_axon_ntff_profile_hook(hook)
        except Exception:
            pass
    from concourse import bass_utils
    if not getattr(bass_utils.upload_artifacts, "_mmd_safe", False):
        orig = bass_utils.upload_artifacts

        def safe_upload(tmpdir):
            try:
                return orig(tmpdir)
            except Exception:
                return tmpdir

        safe_upload._mmd_safe = True
        bass_utils.upload_artifacts = safe_upload


def kernel(X, Y, W1, b1, W2, b2, W3, b3, W4, b4,
           epsilon_opt, sigma_q_opt, sigma_phi_opt):
    global _NC_CACHE, _LAST_RESULT
    import ml_dtypes
    from concourse import bass_utils
    _harden_tracing()

    X = np.ascontiguousarray(np.asarray(X, np.float32))
    Y = np.ascontiguousarray(np.asarray(Y, np.float32))
    W1 = np.ascontiguousarray(np.asarray(W1, np.float32))
    W2 = np.asarray(W2, np.float32)
    W3 = np.asarray(W3, np.float32)
    W4 = np.asarray(W4, np.float64)
    b1 = np.asarray(b1, np.float32)
    b2 = np.asarray(b2, np.float32)
    b3 = np.asarray(b3, np.float32)
    b4 = np.asarray(b4, np.float32)  # cancels exactly in d_feat; unused
    sq = float(np.asarray(sigma_q_opt, np.float64) ** 2)
    sph = float(np.asarray(sigma_phi_opt, np.float64) ** 2)
    eps = float(1.0 / (1.0 + np.exp(-float(np.asarray(epsilon_opt, np.float64)))))
    _ = (b4, eps)  # eps mixture term dropped; see module docstring

    XT = np.ascontiguousarray(X.T)   # [256, 4096]
    YT = np.ascontiguousarray(Y.T)

    # G = W4 W4^T; w-transform lvs = 32*sqrt(2/sph) * L, L = chol(G).
    # The 32x scaling (1024x in the squares; sq = 2048) is exact binary and
    # is undone by the Exp activation's scale = -2/sq.
    G = W4 @ W4.T
    L = np.linalg.cholesky(G)
    lv = (32.0 * np.sqrt(2.0 / sph) * L).astype(np.float32)
    lvstk = np.zeros((SW, 42), np.float32)
    lvstk[0:HID, 0:HID] = lv
    lvstk[64:64 + HID, 32:42] = lv
    w2blk = np.zeros((SW, SW), np.float32)
    w3blk = np.zeros((SW, SW), np.float32)
    b1stk = np.zeros((SW, 1), np.float32)
    b2stk = np.zeros((SW, 1), np.float32)
    b3stk = np.zeros((SW, 1), np.float32)
    for po in (0, 64):
        w2blk[po:po + HID, po:po + HID] = W2
        w3blk[po:po + HID, po:po + HID] = W3
        b1stk[po:po + HID, 0] = b1
        b2stk[po:po + HID, 0] = b2
        b3stk[po:po + HID, 0] = b3
    wsum_ = np.zeros((42, 2), np.float32)
    wsum_[0:HID, 0] = O2048
    wsum_[32:42, 1] = O2048
    common = {
        "w1": W1,
        "w2b": w2blk, "w3b": w3blk, "lvs2": lvstk,
        "b1s2": b1stk, "b2s": b2stk, "b3s": b3stk,
        "wsum": wsum_,
        "onesc": np.ones((128, 1), ml_dtypes.bfloat16),
        "ones2": np.ones((2, BLK), ml_dtypes.bfloat16),
        "eye": np.eye(128, dtype=np.float32),
    }
    perms = []
    in_maps = []
    for c in range(NCORES):
        # cyclic block order: distance-d block at columns [d*512, (d+1)*512)
        perm = np.concatenate(
            [np.arange(((c + d) % NCORES) * BLK, ((c + d) % NCORES) * BLK + BLK)
             for d in range(NCORES)])
        perms.append(perm)
        xtp = np.ascontiguousarray(XT[:, perm])
        ytp = np.ascontiguousarray(YT[:, perm])
        m = dict(common)
        m["xt"] = xtp
        m["yt"] = ytp
        m["xbts"] = np.ascontiguousarray(-xtp[:, :BLK])
        m["ybts"] = np.ascontiguousarray(-ytp[:, :BLK])
        # xon/sq per permuted column in the t128 layout (val[32p+c])
        m["xont"] = ((xtp.astype(np.float64) ** 2).sum(axis=0) / sq
                     ).astype(np.float32).reshape(128, 32)
        m["yont"] = ((ytp.astype(np.float64) ** 2).sum(axis=0) / sq
                     ).astype(np.float32).reshape(128, 32)
        in_maps.append(m)

    if _NC_CACHE is None:
        _NC_CACHE = _build_bass()
    nc = _NC_CACHE

    res = bass_utils.run_bass_kernel_spmd(nc, in_maps, core_ids=list(range(NCORES)))
    _LAST_RESULT = res

    # ---------------- host-side final reduction (float64) ----------------
    # k_x/k_y triangle: sum = s0 + 2*s1 + s2 (d=4 computed from both sides
    # once each); row sums = direct d0..4 rows + incoming transposed column
    # sums from the cores at cyclic distance -1..-3.
    rs_full = {k: np.zeros(N, np.float64) for k in ("x", "y", "z")}
    dg_sum = {k: 0.0 for k in ("x", "y", "z")}
    sum_k = {k: 0.0 for k in ("x", "y", "z")}
    cs_full = np.zeros(N, np.float64)
    for c in range(NCORES):
        out = res.results[c]
        for key, name in (("x", "rsx"), ("y", "rsy")):
            parts = out[name].astype(np.float64)            # [128, NCH*3]
            p3 = parts.reshape(128, NCH, 3)
            rows = p3.sum(axis=2)                           # [128, NCH]
            rs_full[key][c * BLK:(c + 1) * BLK] = rows.T.reshape(BLK)
            sum_k[key] += (p3[:, :, 0].sum() + 2.0 * p3[:, :, 1].sum()
                           + p3[:, :, 2].sum())
        parts = out["rsxy"].astype(np.float64)              # [128, NCH*NST]
        rows = parts.reshape(128, NCH, NST).sum(axis=2)
        rs_full["z"][c * BLK:(c + 1) * BLK] = rows.T.reshape(BLK)
        sum_k["z"] += parts.sum()
        for key, name in (("x", "dgx"), ("y", "dgy"), ("z", "dgxy")):
            dg_sum[key] += float(out[name].astype(np.float64).sum())
        cs_full[perms[c]] += out["csxy"].astype(np.float64)[0]
    for c in range(NCORES):
        for key, name in (("x", "csx"), ("y", "csy")):
            cs3 = res.results[c][name].astype(np.float64)[0]  # [1536]
            for d in (1, 2, 3):
                tgt = ((c + d) % NCORES) * BLK
                rs_full[key][tgt:tgt + BLK] += cs3[(d - 1) * BLK:d * BLK]

    nn1 = float(N) * (N - 1)
    xx = (sum_k["x"] - dg_sum["x"]) / nn1
    yy = (sum_k["y"] - dg_sum["y"]) / nn1
    xy = (sum_k["z"] - dg_sum["z"]) / nn1
    mmd2 = xx - 2.0 * xy + yy

    hs = rs_full["x"] + rs_full["y"] - rs_full["z"] - cs_full
    sum_h = sum_k["x"] + sum_k["y"] - 2.0 * sum_k["z"]
    v1 = (4.0 / N ** 3) * float(hs @ hs)
    v2 = (4.0 / N ** 4) * sum_h ** 2
    var = v1 - v2 + 1e-8
    return np.array([mmd2, var], np.float32)


# revision 55
# speedup vs baseline: 1.3324x; 1.3324x over previous
"""Deep-MMD loss kernel for Trainium2, sharded across 8 NeuronCores.

Strategy (data-parallel row sharding, per the sharding hint):
  - Each core owns a 512-row block of X (and the same-index block of Y) and
    computes its row-blocks of the three 4096x4096 gram matrices
    k_x, k_y, k_xy fully fused on-chip (never materialized to HBM):
        k = exp(-(d_feat/sigma_phi + d_org/sigma_q))
  - Feature distances use the factorization  F = h3 @ W4 (+b4), so
        d_feat = (h3_i - h3_j)^T G (h3_i - h3_j),  G = W4 W4^T  (b4 cancels).
    With L = chol(G), w = 32*sqrt(2/sph)*L^T h3 (10 rows) centered by a
    per-component mean (distances are shift-invariant, so each core may use
    its own mean), the Exp exponent is assembled as
        E_ij = -2/sq * pk_ij - u_i,   pk = -w_i.w_j - x_i.x_j + 1024*u_j
    where u = |w|^2/2048 + xon/sq (= vn + xon/sq, the combined row norm).
    The w cross products run as ONE bf16 K=42 matmul per 512-col strip via
    an error-compensated hi/lo split
        w_i.w_j = hi_i.hi_j + hi_i.lo_j + lo_i.hi_j + lo_i.lo_j
    (all four terms carried -- K is free since matmul cost only depends on
    the streamed columns).  The 32x / 1024x scalings are exact powers of
    two (sq = 2048), undone by the Exp activation's scale=-2/sq; u's 1024x
    enters as an exact bf16 exponent shift on its hi/lo rows.
  - k_x and k_y are symmetric: with a per-core CYCLIC block permutation of
    the columns, each core computes only the 5 blocks at cyclic distance
    d=0..4 (2560 of 4096 cols).  The full sum is s0 + 2*s1(d1..3) + s2(d4)
    and the missing row-sum parts are other cores' transposed column sums
    (csx/csy), assembled on host.  k_xy is not symmetric and runs full.
  - Precision placement (f32r matmul noise measured at ~1.5e-4 relative):
    the MLP, the w-producing matmul, and the |w|^2 row-sum run in exact
    fp32 (2-pass); only the org cross products (O(1) values, tolerance
    ~1e-2 absolute in do) and the xon sums use 1-pass f32r.
  - Row sums fall out of the Exp activation's accum_out for free; the Exp
    writes bf16 so k_xy column sums are a ones-vector matmul with no cast;
    the diagonal (trace) is extracted from the un-rounded PSUM exponent.
  - The eps = sigmoid(epsilon_opt) ~ 5e-11 mixture term contributes
    ~3e-16 to mmd2 (measured in f64) and is dropped.
  - Host (float64) assembles the final [mmd2, var] from per-core partial
    sums ("all-reduce the scalar sums" per the hint).

SPMD trick: every core's column order is cyclically permuted "own block
first" (host-side input prep), so its diagonal always lives in columns
[c*128,(c+1)*128) of the first 512-col block -- the compiled program is
identical on all 8 cores; only input data differs.

Measured: 254906 ns on TRN2 (baseline 674563 ns), rel err 6.5e-4 vs the
f64 oracle (gate 2e-2).
"""

import numpy as np

N = 4096          # samples per side
IN_DIM = 256
HID = 10
NCORES = 8
BLK = N // NCORES           # 512 rows per core
NCH = BLK // 128            # 4 row-chunks of 128 per core
NST = N // 1024             # 4 column supertiles of 1024
SW = 64 + HID               # 74: stacked block 0 at partitions 0:10, block 1 at 64:74
KU = 42                     # U rows: hi lo hi lo (10 each), t_hi, t_lo
NEG2SQ = -1.0 / 1024.0      # -2/sq with sq = 2048 (exact binary)
O2048 = 1.0 / 2048.0        # norm-sum lhs constant (exact binary)


def _build_bass():
    import concourse.bass as bass  # noqa: F401
    import concourse.mybir as mybir
    import concourse.tile as tile
    from concourse import bacc

    f32 = mybir.dt.float32
    f32r = mybir.dt.float32r
    bf16 = mybir.dt.bfloat16
    AFT = mybir.ActivationFunctionType

    nc = bacc.Bacc("TRN2")

    # ---------------- DRAM I/O ----------------
    # One copy of the inputs, declared f32r so the BIR verifier accepts the
    # gram-phase f32r matmul consumers; the MLP reads the same SBUF tiles
    # through a f32 bitcast (the DMA write is a byte copy, so full-precision
    # f32 bits flow to the fp32 matmuls either way).
    xt = nc.dram_tensor("xt", [IN_DIM, N], f32r, kind="ExternalInput")
    yt = nc.dram_tensor("yt", [IN_DIM, N], f32r, kind="ExternalInput")
    xbts = nc.dram_tensor("xbts", [IN_DIM, BLK], f32r, kind="ExternalInput")
    ybts = nc.dram_tensor("ybts", [IN_DIM, BLK], f32r, kind="ExternalInput")
    w1 = nc.dram_tensor("w1", [IN_DIM, HID], f32, kind="ExternalInput")
    w2b = nc.dram_tensor("w2b", [SW, SW], f32, kind="ExternalInput")
    w3b = nc.dram_tensor("w3b", [SW, SW], f32, kind="ExternalInput")
    lvs2 = nc.dram_tensor("lvs2", [SW, 42], f32, kind="ExternalInput")
    b1s2 = nc.dram_tensor("b1s2", [SW, 1], f32, kind="ExternalInput")
    b2s = nc.dram_tensor("b2s", [SW, 1], f32, kind="ExternalInput")
    b3s = nc.dram_tensor("b3s", [SW, 1], f32, kind="ExternalInput")
    wsum = nc.dram_tensor("wsum", [42, 2], f32, kind="ExternalInput")
    # xon/sq per permuted column, host-computed, already in the t128
    # layout (t128[p, c] = val[32p + c])
    xont = nc.dram_tensor("xont", [128, 32], f32, kind="ExternalInput")
    yont = nc.dram_tensor("yont", [128, 32], f32, kind="ExternalInput")
    onesc = nc.dram_tensor("onesc", [128, 1], bf16, kind="ExternalInput")
    ones2 = nc.dram_tensor("ones2", [2, BLK], bf16, kind="ExternalInput")
    eye = nc.dram_tensor("eye", [128, 128], f32, kind="ExternalInput")

    # Triangle outputs for the symmetric k_x/k_y: per row-chunk 3 accum
    # slots (s0 = cyclic-distance-0 block, s1 = d1..d3, s2 = d4), plus the
    # column sums of the d1..d3 blocks for the host's transposed row sums.
    rsx = nc.dram_tensor("rsx", [128, NCH * 3], f32, kind="ExternalOutput")
    rsy = nc.dram_tensor("rsy", [128, NCH * 3], f32, kind="ExternalOutput")
    csx = nc.dram_tensor("csx", [1, 1536], f32, kind="ExternalOutput")
    csy = nc.dram_tensor("csy", [1, 1536], f32, kind="ExternalOutput")
    rsxy = nc.dram_tensor("rsxy", [128, NCH * NST], f32, kind="ExternalOutput")
    csxy = nc.dram_tensor("csxy", [1, N], f32, kind="ExternalOutput")
    dgx = nc.dram_tensor("dgx", [128, NCH], f32, kind="ExternalOutput")
    dgy = nc.dram_tensor("dgy", [128, NCH], f32, kind="ExternalOutput")
    dgxy = nc.dram_tensor("dgxy", [128, NCH], f32, kind="ExternalOutput")

    with tile.TileContext(nc) as tc:
        with tc.tile_pool(name="persist", bufs=1) as pp:
            # ---------- SBUF (persistent) ----------
            t_x = [pp.tile([128, N], f32r, name=f"x{i}", tag=f"x{i}")
                   for i in range(2)]
            t_y = [pp.tile([128, N], f32r, name=f"y{i}", tag=f"y{i}")
                   for i in range(2)]
            t_xbts = [pp.tile([128, BLK], f32r, name=f"xbts{i}", tag=f"xbts{i}")
                      for i in range(2)]
            t_ybts = [pp.tile([128, BLK], f32r, name=f"ybts{i}", tag=f"ybts{i}")
                      for i in range(2)]
            urx = pp.tile([KU, N], bf16, name="urx", tag="urx")
            ury = pp.tile([KU, N], bf16, name="ury", tag="ury")
            ulx = pp.tile([KU, BLK], bf16, name="ulx", tag="ulx")
            uly = pp.tile([KU, BLK], bf16, name="uly", tag="uly")
            t_w1 = [pp.tile([128, HID], f32, name=f"w1{i}", tag=f"w1{i}")
                    for i in range(2)]
            t_w2b = pp.tile([SW, SW], f32, name="w2b", tag="w2b")
            t_w3b = pp.tile([SW, SW], f32, name="w3b", tag="w3b")
            t_lvs2 = pp.tile([SW, 42], f32, name="lvs2", tag="lvs2")
            t_b1s2 = pp.tile([SW, 1], f32, name="b1s2", tag="b1s2")
            t_b2s = pp.tile([SW, 1], f32, name="b2s", tag="b2s")
            t_b3s = pp.tile([SW, 1], f32, name="b3s", tag="b3s")
            t_wsum = pp.tile([42, 2], f32, name="wsum", tag="wsum")
            t_xont = pp.tile([128, 32], f32, name="xont", tag="xont")
            t_yont = pp.tile([128, 32], f32, name="yont", tag="yont")
            t_ones = pp.tile([128, 1], bf16, name="ones", tag="ones")
            t_eye = pp.tile([128, 128], f32, name="eye", tag="eye")
            cnbx = pp.tile([128, NCH], f32, name="cnbx", tag="cnbx")
            cnby = pp.tile([128, NCH], f32, name="cnby", tag="cnby")
            t128x = pp.tile([128, 32], f32, name="t128x", tag="t128x")
            t128y = pp.tile([128, 32], f32, name="t128y", tag="t128y")
            t_rs = {m: pp.tile([128, NCH * 3], f32, name=f"rs{m}", tag=f"rs{m}")
                    for m in "xy"}
            t_rs["z"] = pp.tile([128, NCH * NST], f32, name="rsz", tag="rsz")
            t_dg = {m: pp.tile([128, NCH], f32, name=f"dg{m}", tag=f"dg{m}")
                    for m in "xyz"}

            # ---------- input DMAs ----------
            # Priority order: MLP weights, then x then y chunks (phase B
            # consumes x first); gram-only tensors (xbts, eye, ones) last.
            # The sync queue runs transfers serially, so order = latency.
            for half in range(2):
                nc.sync.dma_start(t_w1[half][:],
                                  w1[half * 128:(half + 1) * 128, :])
            nc.sync.dma_start(t_w2b[:], w2b[:])
            nc.sync.dma_start(t_w3b[:], w3b[:])
            nc.sync.dma_start(t_lvs2[:], lvs2[:])
            nc.sync.dma_start(t_b1s2[:], b1s2[:])
            nc.sync.dma_start(t_b2s[:], b2s[:])
            nc.sync.dma_start(t_b3s[:], b3s[:])
            nc.sync.dma_start(t_wsum[:], wsum[:])
            nc.sync.dma_start(t_xont[:], xont[:])
            nc.sync.dma_start(t_yont[:], yont[:])
            for tt, src in ((t_x, xt), (t_y, yt)):
                for j in range(8):
                    s = slice(j * 512, (j + 1) * 512)
                    for half in range(2):
                        hs_ = slice(half * 128, (half + 1) * 128)
                        nc.sync.dma_start(tt[half][:, s], src[hs_, s])
            for half in range(2):
                hs_ = slice(half * 128, (half + 1) * 128)
                nc.sync.dma_start(t_xbts[half][:], xbts[hs_, :])
                nc.sync.dma_start(t_ybts[half][:], ybts[hs_, :])
            nc.sync.dma_start(t_ones[:], onesc[:])
            nc.sync.dma_start(t_eye[:], eye[:])
            nc.sync.dma_start(ulx[40:42, :], ones2[:])
            nc.sync.dma_start(uly[40:42, :], ones2[:])

            # ---------- Phase B: MLP + w + U assembly + norms ----------
            # softplus(z) = Ln(Exp(z) + 1): no HW softplus table,
            # but ln+exp share one table set.
            if True:
                with tc.tile_pool(name="mlp_ps", bufs=2, space="PSUM") as mps, \
                     tc.tile_pool(name="cn_ps", bufs=2, space="PSUM") as cnps, \
                     tc.tile_pool(name="hp", bufs=1) as hp, \
                     tc.tile_pool(name="ep", bufs=1) as ep, \
                     tc.tile_pool(name="sp", bufs=1) as sp:
                    hh01 = [hp.tile([SW, 2048], f32, name=f"h{l}", tag=f"h{l}")
                            for l in range(2)]
                    t_mneg = sp.tile([42, 1], f32, name="mneg", tag="mneg")
                    for t_in, ur, ul, t128, sname in (
                            (t_x, urx, ulx, t128x, "x"),
                            (t_y, ury, uly, t128y, "y")):
                        # h1 hole rows stay 0 (L2 contracts them against
                        # zero weights; garbage could be NaN)
                        hh = [hh01[0], hh01[1], hh01[0]]  # h3 reuses h1's buf
                        nc.vector.memset(hh[0][:], 0.0)
                        # L1: even blocks -> psum [10,2048] -> h1[0:10,:],
                        #     odd blocks  -> psum [10,2048] -> h1[64:74,:]
                        for par in range(2):
                            for g in range(2):
                                p1 = mps.tile([HID, 1024], f32, name="p1",
                                              tag="mp")
                                for qq in range(2):
                                    b = 2 * (2 * g + qq) + par
                                    s = slice(b * 512, (b + 1) * 512)
                                    po = p1[:, qq * 512:(qq + 1) * 512]
                                    nc.tensor.matmul(po, t_w1[0][:],
                                                     t_in[0][:, s].bitcast(f32),
                                                     start=True, stop=False)
                                    nc.tensor.matmul(po, t_w1[1][:],
                                                     t_in[1][:, s].bitcast(f32),
                                                     start=False, stop=True)
                                dst = hh[0][64 * par:64 * par + HID,
                                            g * 1024:(g + 1) * 1024]
                                e1 = ep.tile([HID, 1024], f32, name="e1",
                                             tag="e1")
                                nc.scalar.activation(e1[:], p1[:], AFT.Exp,
                                                     bias=t_b1s2[0:HID, :])
                                nc.scalar.activation(dst, e1[:], AFT.Ln,
                                                     bias=1.0)
                        # L2, L3: block-diagonal stacked
                        for wt, bt, hsrc, hdst in ((t_w2b, t_b2s, hh[0], hh[1]),
                                                   (t_w3b, t_b3s, hh[1], hh[2])):
                            for g in range(2):
                                pL = mps.tile([SW, 1024], f32, name="pL",
                                              tag="mp")
                                for qq in range(2):
                                    sq_ = slice(g * 1024 + qq * 512,
                                                g * 1024 + qq * 512 + 512)
                                    nc.tensor.matmul(
                                        pL[:, qq * 512:(qq + 1) * 512],
                                        wt[:], hsrc[:, sq_],
                                        start=True, stop=True)
                                eL = ep.tile([SW, 1024], f32, name="ea",
                                             tag="ea")
                                nc.scalar.activation(eL[:], pL[:], AFT.Exp,
                                                     bias=bt[:])
                                nc.scalar.activation(
                                    hdst[:, g * 1024:(g + 1) * 1024], eL[:],
                                    AFT.Ln, bias=1.0)
                        # w = lvs^T @ h3 per q (cols 2q*512 even / odd
                        # stacked as rows 0:10 / 32:42), centered, split
                        # hi/lo bf16, scattered into UR/UL; |w|^2 and xon
                        # sums accumulate u = vn + xon/sq per 512-block.
                        for q in range(4):
                            pv = mps.tile([42, 512], f32, name="pv", tag="mp")
                            nc.tensor.matmul(pv[:], t_lvs2[:],
                                             hh[2][:, q * 512:(q + 1) * 512],
                                             start=True, stop=True)
                            if sname == "x" and q == 0:
                                nc.vector.reduce_sum(
                                    t_mneg[:], pv[:], axis=mybir.AxisListType.X)
                                nc.vector.tensor_scalar_mul(
                                    t_mneg[:], t_mneg[:], 1.0 / 512.0)
                                # both stacked blocks share ONE mean
                                nc.gpsimd.dma_start(t_mneg[32:42, :],
                                                    t_mneg[0:10, :])
                            wq = sp.tile([42, 512], f32, name="wq", tag="wq",
                                         bufs=2)
                            nc.vector.tensor_scalar_sub(wq[:], pv[:], t_mneg[:])
                            hi16 = sp.tile([42, 512], bf16, name="hi16",
                                           tag="hi16", bufs=2)
                            lo16 = sp.tile([42, 512], bf16, name="lo16",
                                           tag="lo16", bufs=2)
                            nc.vector.tensor_copy(hi16[:], wq[:])
                            nc.vector.tensor_sub(lo16[:], wq[:], hi16[:])
                            w2q = sp.tile([42, 512], f32, name="w2q",
                                          tag="w2q", bufs=2)
                            nc.scalar.activation(w2q[:], wq[:], AFT.Square)
                            # vn = |w|^2/2048 for both blocks of this q in
                            # one [2, 512] psum (row 0 even, row 1 odd);
                            # xon/sq is host-provided and added in the tail
                            cnp = cnps.tile([2, 512], f32, name="cnp",
                                            tag="cnp")
                            nc.tensor.matmul(cnp[:], t_wsum[:, 0:2], w2q[:],
                                             start=True, stop=True)
                            cnrow = sp.tile([2, 512], f32, name="cnrow",
                                            tag="cnrow", bufs=2)
                            nc.scalar.copy(cnrow[:], cnp[:])
                            for par in range(2):
                                b = 2 * q + par
                                s = slice(b * 512, (b + 1) * 512)
                                r0 = 32 * par
                                nc.gpsimd.dma_start(ur[0:10, s],
                                                    hi16[r0:r0 + 10, :])
                                nc.gpsimd.dma_start(ur[10:20, s],
                                                    lo16[r0:r0 + 10, :])
                                nc.gpsimd.dma_start(ur[20:30, s],
                                                    hi16[r0:r0 + 10, :])
                                nc.gpsimd.dma_start(ur[30:40, s],
                                                    lo16[r0:r0 + 10, :])
                                if b == 0:
                                    nhi = sp.tile([HID, 512], bf16, name="nhi",
                                                  tag="nhi")
                                    nlo = sp.tile([HID, 512], bf16, name="nlo",
                                                  tag="nlo")
                                    nc.vector.tensor_scalar_mul(
                                        nhi[:], hi16[0:10, :], -1.0)
                                    nc.vector.tensor_scalar_mul(
                                        nlo[:], lo16[0:10, :], -1.0)
                                    nc.gpsimd.dma_start(ul[0:10, :], nhi[:])
                                    nc.gpsimd.dma_start(ul[10:20, :], nhi[:])
                                    nc.gpsimd.dma_start(ul[20:30, :], nlo[:])
                                    nc.gpsimd.dma_start(ul[30:40, :], nlo[:])
                                nc.gpsimd.dma_start(
                                    t128[16 * b:16 * b + 16, :],
                                    cnrow[par:par + 1, :])

            # ---------- Phase C tail: u hi/lo rows + Exp bias ----------
            # t128[p, c] = u[32*p + c]; every row <-> t128 transfer uses the
            # same DMA linearization, so elementwise ops and the chunk
            # extraction (chunk c = t128[4c:4c+4, :]) stay consistent.
            # UR rows carry 1024*u as exact bf16 exponent shifts.
            with tc.tile_pool(name="tp", bufs=1) as tp:
                for sname, ur, t128, t_on, cnb in (
                        ("x", urx, t128x, t_xont, cnbx),
                        ("y", ury, t128y, t_yont, cnby)):
                    usum = tp.tile([128, 32], f32, name="usum", tag="usum")
                    uhi = tp.tile([128, 32], bf16, name="uhi", tag="uhi")
                    uhi32 = tp.tile([128, 32], f32, name="uhi32", tag="uhi32")
                    ulo32 = tp.tile([128, 32], f32, name="ulo32", tag="ulo32")
                    thi = tp.tile([128, 32], bf16, name="thi", tag="thi")
                    tlo = tp.tile([128, 32], bf16, name="tlo", tag="tlo")
                    nc.vector.tensor_add(usum[:], t128[:], t_on[:])
                    nc.vector.tensor_copy(uhi[:], usum[:])
                    nc.vector.tensor_copy(uhi32[:], uhi[:])
                    nc.vector.tensor_sub(ulo32[:], usum[:], uhi32[:])
                    nc.vector.tensor_scalar_mul(thi[:], uhi[:], 1024.0)
                    nc.vector.tensor_scalar_mul(tlo[:], ulo32[:], 1024.0)
                    nc.gpsimd.dma_start(ur[40:41, :], thi[:])
                    nc.gpsimd.dma_start(ur[41:42, :], tlo[:])
                    for c in range(NCH):
                        nc.gpsimd.dma_start(cnb[:, c:c + 1],
                                            usum[4 * c:4 * c + 4, :])
                    nc.vector.tensor_scalar_mul(cnb[:], cnb[:], -1.0)

            # ---------- Phase D1: k_x / k_y triangle (cols 0:2560) ----------
            # Cyclic column permutation means the 5 leading 512-col blocks
            # are cyclic distances d=0..4 from the own row block; d=5..7 are
            # recovered on host from other cores' transposes (k symmetric).
            with tc.tile_pool(name="kp", bufs=2) as kp, \
                 tc.tile_pool(name="gps", bufs=1, space="PSUM") as gps, \
                 tc.tile_pool(name="csbp", bufs=2) as csbp, \
                 tc.tile_pool(name="dp", bufs=3) as dp:
                mats = [
                    ("x", t_xbts, t_x, ulx, urx, cnbx, rsx, dgx, csx),
                    ("y", t_ybts, t_y, uly, ury, cnby, rsy, dgy, csy),
                ]
                for (mkey, bts, ct, ul, ur, cnb, rs_d, dg_d, cs_d) in mats:
                    rs_t, dg_t = t_rs[mkey], t_dg[mkey]
                    csp = gps.tile([1, 1536], f32, name="csp", tag="csp")
                    for c in range(NCH):
                        cs_ = slice(c * 128, (c + 1) * 128)
                        pks = [gps.tile([128, 512], f32, name="pk0", tag="pk0"),
                               gps.tile([128, 1536], f32, name="pk1", tag="pk1"),
                               gps.tile([128, 512], f32, name="pk2", tag="pk2")]
                        for s5 in range(5):
                            ns = slice(s5 * 512, (s5 + 1) * 512)
                            if s5 == 0:
                                po = pks[0][:]
                            elif s5 < 4:
                                po = pks[1][:, (s5 - 1) * 512:s5 * 512]
                            else:
                                po = pks[2][:]
                            nc.tensor.matmul(
                                po, bts[0][:, cs_], ct[0][:, ns],
                                start=True, stop=False)
                            nc.tensor.matmul(
                                po, bts[1][:, cs_], ct[1][:, ns],
                                start=False, stop=False)
                            nc.tensor.matmul(
                                po, ul[:, cs_], ur[:, ns],
                                start=False, stop=True)
                        kts = []
                        for si, pk in enumerate(pks):
                            kt = kp.tile([128, pk.shape[1]], bf16,
                                         name=f"kt{si}", tag=f"kt{si}")
                            nc.scalar.activation(
                                kt[:], pk[:], AFT.Exp, scale=NEG2SQ,
                                bias=cnb[:, c:c + 1],
                                accum_out=rs_t[:, c * 3 + si:c * 3 + si + 1])
                            kts.append(kt)
                        for k3 in range(3):
                            nc.tensor.matmul(
                                csp[:, k3 * 512:(k3 + 1) * 512],
                                t_ones[:],
                                kts[1][:, k3 * 512:(k3 + 1) * 512],
                                start=(c == 0), stop=(c == NCH - 1))
                        # diag from the un-rounded PSUM exponent (own block)
                        dtmp = dp.tile([128, 128], f32, name="dtmp", tag="dtmp")
                        ez = dp.tile([128, 1], f32, name="ez", tag="ez")
                        nc.vector.tensor_mul(dtmp[:], pks[0][:, cs_], t_eye[:])
                        nc.vector.reduce_sum(
                            ez[:], dtmp[:], axis=mybir.AxisListType.X)
                        nc.scalar.activation(
                            dg_t[:, c:c + 1], ez[:], AFT.Exp,
                            scale=NEG2SQ, bias=cnb[:, c:c + 1])
                    csrow = csbp.tile([1, 1536], f32, name="csrow", tag="csrow")
                    nc.scalar.copy(csrow[:], csp[:])
                    nc.sync.dma_start(cs_d[:], csrow[:])
                    nc.sync.dma_start(rs_d[:], rs_t[:])
                    nc.sync.dma_start(dg_d[:], dg_t[:])

            # ---------- Phase D2: k_xy (full 4096 cols) ----------
            with tc.tile_pool(name="kpz", bufs=4) as kpz, \
                 tc.tile_pool(name="gpsz", bufs=3, space="PSUM") as gpsz, \
                 tc.tile_pool(name="csps", bufs=1, space="PSUM") as csps, \
                 tc.tile_pool(name="csbpz", bufs=2) as csbpz, \
                 tc.tile_pool(name="dpz", bufs=3) as dpz:
                rs_t, dg_t = t_rs["z"], t_dg["z"]
                for j in range(NST):
                    csp = csps.tile([1, 1024], f32, name="cspz", tag="cspz")
                    for c in range(NCH):
                        cs_ = slice(c * 128, (c + 1) * 128)
                        pk = gpsz.tile([128, 1024], f32, name="pk", tag="pk")
                        for nh in range(2):
                            ns = slice(j * 1024 + nh * 512,
                                       j * 1024 + nh * 512 + 512)
                            po = pk[:, nh * 512:(nh + 1) * 512]
                            nc.tensor.matmul(
                                po, t_xbts[0][:, cs_], t_y[0][:, ns],
                                start=True, stop=False)
                            nc.tensor.matmul(
                                po, t_xbts[1][:, cs_], t_y[1][:, ns],
                                start=False, stop=False)
                            nc.tensor.matmul(
                                po, ulx[:, cs_], ury[:, ns],
                                start=False, stop=True)
                        kt = kpz.tile([128, 1024], bf16, name="kt", tag="kt")
                        nc.scalar.activation(
                            kt[:], pk[:], AFT.Exp, scale=NEG2SQ,
                            bias=cnbx[:, c:c + 1],
                            accum_out=rs_t[:, c * NST + j:c * NST + j + 1])
                        for nh in range(2):
                            nc.tensor.matmul(
                                csp[:, nh * 512:(nh + 1) * 512],
                                t_ones[:],
                                kt[:, nh * 512:(nh + 1) * 512],
                                start=(c == 0), stop=(c == NCH - 1))
                        if j == 0:
                            dtmp = dpz.tile([128, 128], f32, name="dtmp",
                                            tag="dtmp")
                            ez = dpz.tile([128, 1], f32, name="ez", tag="ez")
                            nc.vector.tensor_mul(dtmp[:], pk[:, cs_], t_eye[:])
                            nc.vector.reduce_sum(
                                ez[:], dtmp[:], axis=mybir.AxisListType.X)
                            nc.scalar.activation(
                                dg_t[:, c:c + 1], ez[:], AFT.Exp,
                                scale=NEG2SQ, bias=cnbx[:, c:c + 1])
                    csrow = csbpz.tile([1, 1024], f32, name="csrowz",
                                       tag="csrowz")
                    nc.scalar.copy(csrow[:], csp[:])
                    nc.sync.dma_start(
                        csxy[:, j * 1024:(j + 1) * 1024], csrow[:])
                nc.sync.dma_start(rsxy[:], rs_t[:])
                nc.sync.dma_start(dgxy[:], dg_t[:])

    # Force a single activation table set (ln+exp+square+copy all live in
    # natural_log_exp_and_others); the default greedy choice ping-pongs
    # between exp_and_others and natural_log, costing a ~1.3us table load
    # per switch.
    tabs = bacc.get_activation_tables(nc.m.arch)
    only = {name: (funcs if name == "natural_log_exp_and_others" else set())
            for name, funcs in tabs.items()}
    orig_fn = bacc.get_activation_tables
    bacc.get_activation_tables = lambda arch: only
    try:
        nc.compile()
    finally:
        bacc.get_activation_tables = orig_fn
    return nc


_NC_CACHE = None
_LAST_RESULT = None


def _harden_tracing():
    """Make run_bass_kernel_spmd(trace=True / BASS_TRACE=1) survive in
    containers whose antenv package lacks axon_hooks, and whose bucket
    upload is unavailable. No-ops when everything is present."""
    import sys
    import types
    try:
        import antenv.axon_hooks  # noqa: F401
    except ImportError:
        mod = types.ModuleType("antenv.axon_hooks")
        mod._hook = None
        mod.set_axon_ntff_profile_hook = lambda h: setattr(mod, "_hook", h)
        mod.get_axon_ntff_profile_hook = lambda: mod._hook
        sys.modules["antenv.axon_hooks"] = mod
        try:
            import antenv
            antenv.axon_hooks = mod
        except ImportError:
            pass
        try:
            from trn_agent_boot.trn_boot import _ntff_profile_via_ctypes
            hook = _ntff_profile_via_ctypes("/opt/axon/libaxon_pjrt.so")
            if hook is not None:
                mod.set_axon_ntff_profile_hook(hook)
        except Exception:
            pass
    from concourse import bass_utils
    if not getattr(bass_utils.upload_artifacts, "_mmd_safe", False):
        orig = bass_utils.upload_artifacts

        def safe_upload(tmpdir):
            try:
                return orig(tmpdir)
            except Exception:
                return tmpdir

        safe_upload._mmd_safe = True
        bass_utils.upload_artifacts = safe_upload


def kernel(X, Y, W1, b1, W2, b2, W3, b3, W4, b4,
           epsilon_opt, sigma_q_opt, sigma_phi_opt):
    global _NC_CACHE, _LAST_RESULT
    import ml_dtypes
    from concourse import bass_utils
    _harden_tracing()

    X = np.ascontiguousarray(np.asarray(X, np.float32))
    Y = np.ascontiguousarray(np.asarray(Y, np.float32))
    W1 = np.ascontiguousarray(np.asarray(W1, np.float32))
    W2 = np.asarray(W2, np.float32)
    W3 = np.asarray(W3, np.float32)
    W4 = np.asarray(W4, np.float64)
    b1 = np.asarray(b1, np.float32)
    b2 = np.asarray(b2, np.float32)
    b3 = np.asarray(b3, np.float32)
    b4 = np.asarray(b4, np.float32)  # cancels exactly in d_feat; unused
    sq = float(np.asarray(sigma_q_opt, np.float64) ** 2)
    sph = float(np.asarray(sigma_phi_opt, np.float64) ** 2)
    eps = float(1.0 / (1.0 + np.exp(-float(np.asarray(epsilon_opt, np.float64)))))
    _ = (b4, eps)  # eps mixture term dropped; see module docstring

    XT = np.ascontiguousarray(X.T)   # [256, 4096]
    YT = np.ascontiguousarray(Y.T)

    # G = W4 W4^T; w-transform lvs = 32*sqrt(2/sph) * L, L = chol(G).
    # The 32x scaling (1024x in the squares; sq = 2048) is exact binary and
    # is undone by the Exp activation's scale = -2/sq.
    G = W4 @ W4.T
    L = np.linalg.cholesky(G)
    lv = (32.0 * np.sqrt(2.0 / sph) * L).astype(np.float32)
    lvstk = np.zeros((SW, 42), np.float32)
    lvstk[0:HID, 0:HID] = lv
    lvstk[64:64 + HID, 32:42] = lv
    w2blk = np.zeros((SW, SW), np.float32)
    w3blk = np.zeros((SW, SW), np.float32)
    b1stk = np.zeros((SW, 1), np.float32)
    b2stk = np.zeros((SW, 1), np.float32)
    b3stk = np.zeros((SW, 1), np.float32)
    for po in (0, 64):
        w2blk[po:po + HID, po:po + HID] = W2
        w3blk[po:po + HID, po:po + HID] = W3
        b1stk[po:po + HID, 0] = b1
        b2stk[po:po + HID, 0] = b2
        b3stk[po:po + HID, 0] = b3
    wsum_ = np.zeros((42, 2), np.float32)
    wsum_[0:HID, 0] = O2048
    wsum_[32:42, 1] = O2048
    common = {
        "w1": W1,
        "w2b": w2blk, "w3b": w3blk, "lvs2": lvstk,
        "b1s2": b1stk, "b2s": b2stk, "b3s": b3stk,
        "wsum": wsum_,
        "onesc": np.ones((128, 1), ml_dtypes.bfloat16),
        "ones2": np.ones((2, BLK), ml_dtypes.bfloat16),
        "eye": np.eye(128, dtype=np.float32),
    }
    perms = []
    in_maps = []
    for c in range(NCORES):
        # cyclic block order: distance-d block at columns [d*512, (d+1)*512)
        perm = np.concatenate(
            [np.arange(((c + d) % NCORES) * BLK, ((c + d) % NCORES) * BLK + BLK)
             for d in range(NCORES)])
        perms.append(perm)
        xtp = np.ascontiguousarray(XT[:, perm])
        ytp = np.ascontiguousarray(YT[:, perm])
        m = dict(common)
        m["xt"] = xtp
        m["yt"] = ytp
        m["xbts"] = np.ascontiguousarray(-xtp[:, :BLK])
        m["ybts"] = np.ascontiguousarray(-ytp[:, :BLK])
        # xon/sq per permuted column in the t128 layout (val[32p+c])
        m["xont"] = ((xtp.astype(np.float64) ** 2).sum(axis=0) / sq
                     ).astype(np.float32).reshape(128, 32)
        m["yont"] = ((ytp.astype(np.float64) ** 2).sum(axis=0) / sq
                     ).astype(np.float32).reshape(128, 32)
        in_maps.append(m)

    if _NC_CACHE is None:
        _NC_CACHE = _build_bass()
    nc = _NC_CACHE

    res = bass_utils.run_bass_kernel_spmd(nc, in_maps, core_ids=list(range(NCORES)))
    _LAST_RESULT = res

    # ---------------- host-side final reduction (float64) ----------------
    # k_x/k_y triangle: sum = s0 + 2*s1 + s2 (d=4 computed from both sides
    # once each); row sums = direct d0..4 rows + incoming transposed column
    # sums from the cores at cyclic distance -1..-3.
    rs_full = {k: np.zeros(N, np.float64) for k in ("x", "y", "z")}
    dg_sum = {k: 0.0 for k in ("x", "y", "z")}
    sum_k = {k: 0.0 for k in ("x", "y", "z")}
    cs_full = np.zeros(N, np.float64)
    for c in range(NCORES):
        out = res.results[c]
        for key, name in (("x", "rsx"), ("y", "rsy")):
            parts = out[name].astype(np.float64)            # [128, NCH*3]
            p3 = parts.reshape(128, NCH, 3)
            rows = p3.sum(axis=2)                           # [128, NCH]
            rs_full[key][c * BLK:(c + 1) * BLK] = rows.T.reshape(BLK)
            sum_k[key] += (p3[:, :, 0].sum() + 2.0 * p3[:, :, 1].sum()
                           + p3[:, :, 2].sum())
        parts = out["rsxy"].astype(np.float64)              # [128, NCH*NST]
        rows = parts.reshape(128, NCH, NST).sum(axis=2)
        rs_full["z"][c * BLK:(c + 1) * BLK] = rows.T.reshape(BLK)
        sum_k["z"] += parts.sum()
        for key, name in (("x", "dgx"), ("y", "dgy"), ("z", "dgxy")):
            dg_sum[key] += float(out[name].astype(np.float64).sum())
        cs_full[perms[c]] += out["csxy"].astype(np.float64)[0]
    for c in range(NCORES):
        for key, name in (("x", "csx"), ("y", "csy")):
            cs3 = res.results[c][name].astype(np.float64)[0]  # [1536]
            for d in (1, 2, 3):
                tgt = ((c + d) % NCORES) * BLK
                rs_full[key][tgt:tgt + BLK] += cs3[(d - 1) * BLK:d * BLK]

    nn1 = float(N) * (N - 1)
    xx = (sum_k["x"] - dg_sum["x"]) / nn1
    yy = (sum_k["y"] - dg_sum["y"]) / nn1
    xy = (sum_k["z"] - dg_sum["z"]) / nn1
    mmd2 = xx - 2.0 * xy + yy

    hs = rs_full["x"] + rs_full["y"] - rs_full["z"] - cs_full
    sum_h = sum_k["x"] + sum_k["y"] - 2.0 * sum_k["z"]
    v1 = (4.0 / N ** 3) * float(hs @ hs)
    v2 = (4.0 / N ** 4) * sum_h ** 2
    var = v1 - v2 + 1e-8
    return np.array([mmd2, var], np.float32)
